# revision 24
# baseline (speedup 1.0000x reference)
"""Block-sparse attention (sliding window of 3 x 64-token blocks) on 8 trn2 cores.

Problem: B=1, H=16, S=4096, D=128, fp32 I/O. Token i attends to token j iff
|i//64 - j//64| <= 1, i.e. a 192-key window per 64-query block.

Sharding: head-parallel — 2 heads per NeuronCore, no cross-core traffic.

Per-core kernel (per head):
  - Host packs, per head, ONE fp16 input tensor in consumption order:
    8 chunks, each [qT cols | kT cols | augmented-V cols], so one DMA per
    chunk streams everything the next group of q-tiles needs.
      qT: Q^T [d=128, S]
      kT: K^T zero-padded by 64 keys on each end [d=128, S+128]
      va: V augmented with a ones-column, rearranged to [128, 33*129] so
          each 128-key chunk c lives at cols [129c, 129c+129)
  - Work unit is a GROUP of 4 query tiles (512 queries, tiles 4g..4g+3).
    For tile t the allowed keys are the padded window [128t, 128t+256):
    key chunks t and t+1 — so a group needs chunks 4g..4g+4, and interior
    chunks serve two adjacent tiles.
  - MM1 (PE): chunk-ordered into ps [128, 1024] fp32 (2 PSUM banks), layout
    [A_t0|B_t0|A_t1|B_t1|...]: interior chunks do ONE 256-col matmul
    (rhs = two adjacent q-tiles, same stationary kT chunk); the chunk at
    the bank boundary splits in two. 6 matmuls/group instead of 8.
  - ACT: ONE exp over [128, 1024] (cross-bank PSUM read) -> fp16 P.
    (No max-subtraction: scores*scale in (-7.5, 7.5), exp < 1724, fp16-safe;
    exact softmax up to rounding.)
  - GPSIMD: memset the disallowed 64x64 corners to 0 (strided APs, two
    memsets per group; GpSimd is otherwise idle, DVE is cast-loaded).
  - MM2 (PE): per tile, psO [q=128, 129] = P_A^T.T @ VA_A + P_B^T.T @ VA_B
    accumulated in PSUM; col 128 (ones-column) accumulates the softmax
    denominator for free.
  - DVE: copy psO -> fp16 SBUF (max |PV| ~5.8e3, max den ~2.2e3: fp16-safe).
    Normalization division happens on the HOST: out = PV/den.
  - Output written as [128, 16 pairs * 258] fp16; host divides+reassembles.
  - All DMAs trigger on the Sync (SP) HWDGE ring: input triggers have
    drained by the time outputs start, and Activation-ring triggers would
    block the exp stream on the Scalar sequencer FIFO.

Emission is software-pipelined by group at DEPTH=2: groups n+1 and n+2's
MM1 blocks are emitted before group n's exp/memset/MM2/copy, so next-group
MM1s never sit behind current-group MM2s in the PE FIFO and the ACT engine
stays fed (3 ps score buffers + 2 po accumulator banks = all 8 PSUM banks).
"""

import bisect
import math

import numpy as np

B, H, S, D = 1, 16, 4096, 128
N_CORES = 8
HPC = H // N_CORES          # heads per core
TILE = 128
NT = S // TILE              # 32 query tiles per head
NG = NT // 4                # 8 groups of 4 tiles per head
PAD = 64
SPAD = S + 2 * PAD          # 4224 padded keys
NCHUNK = SPAD // TILE       # 33 key chunks
VAW = NCHUNK * (D + 1)      # 4257 cols of rearranged augmented V
SCALE = 1.0 / math.sqrt(D)

# Packed-input chunking (consumption order). Boundaries are 512-aligned (qt),
# 128-aligned (kt) and 129-aligned (va) so every kernel slice stays inside
# one segment.
_T = [0, 4, 8, 12, 16, 20, 24, 28, 32]   # q-tile boundaries per chunk
QT_B = [128 * t for t in _T]
KT_B = [0] + [128 * t + 256 for t in _T[1:-1]] + [SPAD]
VA_B = [0] + [129 * (t + 1) for t in _T[1:-1]] + [VAW]
NCK = len(QT_B) - 1
QT_W = [QT_B[i + 1] - QT_B[i] for i in range(NCK)]
KT_W = [KT_B[i + 1] - KT_B[i] for i in range(NCK)]
VA_W = [VA_B[i + 1] - VA_B[i] for i in range(NCK)]
CHUNK_W = [QT_W[i] + KT_W[i] + VA_W[i] for i in range(NCK)]
BASE = [0]
for i in range(NCK):
    BASE.append(BASE[-1] + CHUNK_W[i])
W_PACK = BASE[-1]

_PROGRAM = None


def _qt_off(x):
    i = bisect.bisect_right(QT_B, x) - 1
    return BASE[i] + (x - QT_B[i]), i


def _kt_off(y):
    i = bisect.bisect_right(KT_B, y) - 1
    return BASE[i] + QT_W[i] + (y - KT_B[i]), i


def _va_off(z):
    i = bisect.bisect_right(VA_B, z) - 1
    return BASE[i] + QT_W[i] + KT_W[i] + (z - VA_B[i]), i


def _build_program():
    from contextlib import ExitStack

    import concourse.mybir as mybir
    import concourse.tile as tile
    from concourse import bacc

    f16 = mybir.dt.float16
    f32 = mybir.dt.float32
    Exp = mybir.ActivationFunctionType.Exp

    nc = bacc.Bacc("TRN2", target_bir_lowering=False, debug=False)
    qkv_d = nc.declare_dram_parameter("qkv", [HPC, 128, W_PACK], f16, isOutput=False)
    out_d = nc.declare_dram_parameter("out", [HPC, 128, NT // 2 * 258], f16, isOutput=True)

    def qt_sl(sb, x0, w):
        off, i = _qt_off(x0)
        assert x0 + w <= QT_B[i + 1], (x0, w)
        return sb[:, off:off + w]

    def kt_sl(sb, y0, w):
        off, i = _kt_off(y0)
        assert y0 + w <= KT_B[i + 1], (y0, w)
        return sb[:, off:off + w]

    def va_sl(sb, z0, w):
        off, i = _va_off(z0)
        assert z0 + w <= VA_B[i + 1], (z0, w)
        return sb[:, off:off + w]

    with tile.TileContext(nc) as tc, ExitStack() as ctx:
        io_pool = ctx.enter_context(tc.tile_pool(name="io", bufs=2))
        out_pool = ctx.enter_context(tc.tile_pool(name="outp", bufs=2))
        p_pool = ctx.enter_context(tc.tile_pool(name="p", bufs=4))
        ps_pool = ctx.enter_context(tc.tile_pool(name="ps", bufs=3, space="PSUM"))
        po_pool = ctx.enter_context(tc.tile_pool(name="po", bufs=2, space="PSUM"))

        # Group (0,0)'s score tile doubles as the warmup/touch scratch target:
        # its first real MM1 (start=True) clears whatever the scratch writes
        # left behind, so no dedicated PSUM bank is burned on scratch.
        ps00 = ps_pool.tile([128, 1024], f32, tag="ps")

        # PE warmup sized to fit inside the pre-data window: sustained PE
        # activity flips the HAM clock gate to 2.4 GHz early and reliably.
        warm_pool = ctx.enter_context(tc.tile_pool(name="warm", bufs=1))
        warm = warm_pool.tile([128, 512], f16, tag="warm")
        nc.gpsimd.memset(warm[:], 0.0)
        for _ in range(4):
            nc.tensor.matmul(
                ps00[0:1, 0:512], lhsT=warm[:, 0:1], rhs=warm[:],
                start=True, stop=True,
            )

        # Load phase: ALL input DMAs (both heads) are emitted first so they
        # outrank output DMAs in scheduler priority.
        io_sbs = []
        for h in range(HPC):
            io_sb = io_pool.tile([128, W_PACK], f16, tag="io")
            io_sbs.append(io_sb)
        for h in range(HPC):
            io_sb = io_sbs[h]
            for c in range(NCK):
                nc.sync.dma_start(
                    io_sb[:, BASE[c]:BASE[c + 1]], qkv_d[h, :, BASE[c]:BASE[c + 1]]
                )

        # Compute phase, software-pipelined EMISSION order by group of 4
        # tiles: group n+1's MM1 block is emitted before group n's
        # exp/memset/MM2/copy so the PE produces scores one group ahead and
        # the ACT engine never starves.
        groups = [(h, g) for h in range(HPC) for g in range(NG)]
        out_sbs = {}
        ps_tiles = {}

        def emit_mm1(h, g):
            io_sb = io_sbs[h]
            if (h, g) == (0, 0):
                ps = ps00
            else:
                ps = ps_pool.tile([128, 1024], f32, tag="ps")
            ps_tiles[(h, g)] = ps
            t0 = 4 * g
            # chunk-ordered: interior chunks do one 256-col matmul over two
            # adjacent q-tiles with the same stationary kT chunk; the chunk
            # at the PSUM bank boundary (4g+2) splits into two 128-col MMs.
            nc.tensor.matmul(
                ps[:, 0:128], lhsT=kt_sl(io_sb, 128 * t0, 128),
                rhs=qt_sl(io_sb, 128 * t0, 128), start=True, stop=True,
            )
            nc.tensor.matmul(
                ps[:, 128:384], lhsT=kt_sl(io_sb, 128 * (t0 + 1), 128),
                rhs=qt_sl(io_sb, 128 * t0, 256), start=True, stop=True,
            )
            nc.tensor.matmul(
                ps[:, 384:512], lhsT=kt_sl(io_sb, 128 * (t0 + 2), 128),
                rhs=qt_sl(io_sb, 128 * (t0 + 1), 128), start=True, stop=True,
            )
            nc.tensor.matmul(
                ps[:, 512:640], lhsT=kt_sl(io_sb, 128 * (t0 + 2), 128),
                rhs=qt_sl(io_sb, 128 * (t0 + 2), 128), start=True, stop=True,
            )
            nc.tensor.matmul(
                ps[:, 640:896], lhsT=kt_sl(io_sb, 128 * (t0 + 3), 128),
                rhs=qt_sl(io_sb, 128 * (t0 + 2), 256), start=True, stop=True,
            )
            nc.tensor.matmul(
                ps[:, 896:1024], lhsT=kt_sl(io_sb, 128 * (t0 + 4), 128),
                rhs=qt_sl(io_sb, 128 * (t0 + 3), 128), start=True, stop=True,
            )
            # PE "touch" of the NEXT packing segment, emitted AFTER this
            # group's real MM1s so they don't wait on segment g+1: a 1-col
            # dummy matmul makes PE observe that segment's DMA semaphore,
            # keeping group g+1's matmuls within the 2-sync-wait HW limit
            # without a dedicated scratch bank. It lands in ps[0, 64] --
            # inside tile t0's disallowed A-corner, which the post-exp
            # memset zeroes -- so the garbage never reaches MM2. (Its
            # start=True clears bank-0 has_written bits, which is harmless:
            # no later matmul accumulates into this ps tile.)
            if g + 1 < NCK:
                b1 = BASE[g + 1]
                nc.tensor.matmul(
                    ps[0:1, 64:65], lhsT=io_sb[:, b1:b1 + 1],
                    rhs=io_sb[:, b1:b1 + 1], start=True, stop=True,
                )

        def emit_tail(h, g):
            io_sb = io_sbs[h]
            out_sb = out_sbs[h]
            ps = ps_tiles.pop((h, g))
            p_sb = p_pool.tile([128, 1024], f16, tag="p")
            nc.scalar.activation(p_sb[:], ps[:], Exp, bias=0.0, scale=SCALE)
            # Kill disallowed 64x64 corners (cols 256*tl+0:128 = chunk A of
            # tile t, 256*tl+128:256 = chunk B); boundary tiles kill the
            # whole 64-row pad block instead. One strided memset covers all
            # four tiles' A-corners, another the four B-corners, so the
            # GpSimd stream never paces the MM2 stream.
            p3 = p_sb[:].rearrange("p (t w) -> p t w", t=4)
            nc.gpsimd.memset(p3[0:64, :, 64:128], 0.0)
            nc.gpsimd.memset(p3[64:128, :, 128:192], 0.0)
            if g == 0:
                nc.gpsimd.memset(p_sb[0:64, 0:64], 0.0)      # tile 0 pad block
            if g == NG - 1:
                nc.gpsimd.memset(p_sb[64:128, 960:1024], 0.0)  # last tile pad
            po_a = po_pool.tile([128, 258], f32, tag="po")
            po_b = po_pool.tile([128, 258], f32, tag="po")
            pos = [po_a, po_b]
            for tl in range(4):
                t = 4 * g + tl
                po = pos[tl // 2]
                o0 = 129 * (tl % 2)
                nc.tensor.matmul(
                    po[:, o0:o0 + 129],
                    lhsT=p_sb[:, 256 * tl:256 * tl + 128],
                    rhs=va_sl(io_sb, 129 * t, 129),
                    start=True, stop=False,
                )
                nc.tensor.matmul(
                    po[:, o0:o0 + 129],
                    lhsT=p_sb[:, 256 * tl + 128:256 * tl + 256],
                    rhs=va_sl(io_sb, 129 * (t + 1), 129),
                    start=False, stop=True,
                )
            for i in range(2):
                u = 2 * g + i
                nc.vector.tensor_copy(out_sb[:, u * 258:(u + 1) * 258], pos[i][:])
            # Stream the output back as groups complete; the final chunk is
            # small so the trailing DMA after the last pair's compute is
            # short. Activation-ring triggers: measured faster than sharing
            # the Sync ring with the input-trigger stream.
            # Head 0's outputs are deferred until g==6, by which time the
            # input stream has drained -- an early g==3 output would steal
            # input DMA bandwidth mid-stream. The last head keeps the final
            # chunks small so the trailing DMA after the last pair's
            # compute is short.
            last = h == HPC - 1
            if not last:
                if g == 6:
                    c0, c1 = 0, 14 * 258
                    nc.sync.dma_start(out_d[h, :, c0:c1], out_sb[:, c0:c1])
                elif g == 7:
                    c0, c1 = 14 * 258, 16 * 258
                    nc.sync.dma_start(out_d[h, :, c0:c1], out_sb[:, c0:c1])
            else:
                if g == 3:
                    c0, c1 = 0, 8 * 258
                    nc.sync.dma_start(out_d[h, :, c0:c1], out_sb[:, c0:c1])
                elif g == 6:
                    c0, c1 = 8 * 258, 14 * 258
                    nc.sync.dma_start(out_d[h, :, c0:c1], out_sb[:, c0:c1])
                elif g == 7:
                    c0, c1 = 14 * 258, 15 * 258
                    nc.sync.dma_start(out_d[h, :, c0:c1], out_sb[:, c0:c1])
                    c0, c1 = 15 * 258, 16 * 258
                    nc.sync.dma_start(out_d[h, :, c0:c1], out_sb[:, c0:c1])

        DEPTH = 2
        for n in range(len(groups) + DEPTH):
            if n < len(groups):
                h, g = groups[n]
                if g == 0:
                    out_sb = out_pool.tile([128, NT // 2 * 258], f16, tag="out")
                    out_sbs[h] = out_sb
                emit_mm1(h, g)
            if n >= DEPTH:
                emit_tail(*groups[n - DEPTH])

    nc.finalize()
    return nc


def _get_program():
    global _PROGRAM
    if _PROGRAM is None:
        _PROGRAM = _build_program()
    return _PROGRAM


def _pack_inputs(q, k, v):
    """q,k,v: [H, S, D] fp32 -> packed [H, 128, W_PACK] fp16 per head."""
    qt = np.ascontiguousarray(q.transpose(0, 2, 1)).astype(np.float16)  # [H,128,S]
    k_pad = np.zeros((H, SPAD, D), np.float32)
    k_pad[:, PAD:PAD + S] = k
    kt = np.ascontiguousarray(k_pad.transpose(0, 2, 1)).astype(np.float16)
    v_aug = np.zeros((H, SPAD, D + 1), np.float32)
    v_aug[:, PAD:PAD + S, :D] = v
    v_aug[:, :, D] = 1.0
    va = np.ascontiguousarray(
        v_aug.reshape(H, NCHUNK, 128, D + 1).transpose(0, 2, 1, 3)
    ).reshape(H, 128, VAW).astype(np.float16)
    segs = []
    for c in range(NCK):
        segs.append(qt[:, :, QT_B[c]:QT_B[c + 1]])
        segs.append(kt[:, :, KT_B[c]:KT_B[c + 1]])
        segs.append(va[:, :, VA_B[c]:VA_B[c + 1]])
    return np.ascontiguousarray(np.concatenate(segs, axis=2))


def kernel(q, k, v):
    """q, k, v: [1, 16, 4096, 128] float32 -> [1, 16, 4096, 128] float32."""
    from concourse.bass_utils import run_bass_kernel_spmd

    q = np.asarray(q, dtype=np.float32).reshape(H, S, D)
    k = np.asarray(k, dtype=np.float32).reshape(H, S, D)
    v = np.asarray(v, dtype=np.float32).reshape(H, S, D)

    qkv = _pack_inputs(q, k, v)
    in_maps = [
        {"qkv": np.ascontiguousarray(qkv[c * HPC:(c + 1) * HPC])}
        for c in range(N_CORES)
    ]

    nc = _get_program()
    results = run_bass_kernel_spmd(nc, in_maps, list(range(N_CORES))).results

    out = np.empty((H, S, D), np.float32)
    for c in range(N_CORES):
        o = results[c]["out"]  # [HPC, 128, 16*258] fp16: per pair [PV_A|den_A|PV_B|den_B]
        for j in range(HPC):
            x = o[j].astype(np.float32).reshape(128, NT, D + 1)  # [p, t, 129]
            pv = x[:, :, :D] / x[:, :, D:D + 1]     # normalize on host
            out[c * HPC + j] = pv.transpose(1, 0, 2).reshape(S, D)
    return out.reshape(B, H, S, D)


# revision 26
# speedup vs baseline: 1.0614x; 1.0614x over previous
"""Block-sparse attention (sliding window of 3 x 64-token blocks) on 8 trn2 cores.

Problem: B=1, H=16, S=4096, D=128, fp32 I/O. Token i attends to token j iff
|i//64 - j//64| <= 1, i.e. a 192-key window per 64-query block.

Sharding: head-parallel — 2 heads per NeuronCore, no cross-core traffic.

Per-core kernel (per head):
  - Host packs, per head, ONE fp16 input tensor in consumption order:
    8 chunks, each [qT cols | kT cols | augmented-V cols], so one DMA per
    chunk streams everything the next group of q-tiles needs.
      qT: Q^T [d=128, S]
      kT: K^T zero-padded by 64 keys on each end [d=128, S+128]
      va: V augmented with a ones-column, rearranged to [128, 33*129] so
          each 128-key chunk c lives at cols [129c, 129c+129)
  - Work unit is a GROUP of 4 query tiles (512 queries, tiles 4g..4g+3).
    For tile t the allowed keys are the padded window [128t, 128t+256):
    key chunks t and t+1 — so a group needs chunks 4g..4g+4, and interior
    chunks serve two adjacent tiles.
  - MM1 (PE): chunk-ordered into ps [128, 1024] fp32 (2 PSUM banks), layout
    [A_t0|B_t0|A_t1|B_t1|...]: interior chunks do ONE 256-col matmul
    (rhs = two adjacent q-tiles, same stationary kT chunk); the chunk at
    the bank boundary splits in two. 6 matmuls/group instead of 8.
  - ACT: ONE exp over [128, 1024] (cross-bank PSUM read) -> fp16 P.
    (No max-subtraction: scores*scale in (-7.5, 7.5), exp < 1724, fp16-safe;
    exact softmax up to rounding.)
  - GPSIMD: memset the disallowed 64x64 corners to 0 (strided APs, two
    memsets per group; GpSimd is otherwise idle, DVE is cast-loaded).
  - MM2 (PE): per tile, psO [q=128, 129] = P_A^T.T @ VA_A + P_B^T.T @ VA_B
    accumulated in PSUM; col 128 (ones-column) accumulates the softmax
    denominator for free.
  - DVE: copy psO -> fp16 SBUF (max |PV| ~5.8e3, max den ~2.2e3: fp16-safe).
    Normalization division happens on the HOST: out = PV/den.
  - Output written as [128, 16 pairs * 258] fp16; host divides+reassembles.
  - All DMAs trigger on the Sync (SP) HWDGE ring: input triggers have
    drained by the time outputs start, and Activation-ring triggers would
    block the exp stream on the Scalar sequencer FIFO.

Emission is software-pipelined by group at DEPTH=2: groups n+1 and n+2's
MM1 blocks are emitted before group n's exp/memset/MM2/copy, so next-group
MM1s never sit behind current-group MM2s in the PE FIFO and the ACT engine
stays fed (3 ps score buffers + 2 po accumulator banks = all 8 PSUM banks).
"""

import bisect
import math

import numpy as np

B, H, S, D = 1, 16, 4096, 128
N_CORES = 8
HPC = H // N_CORES          # heads per core
TILE = 128
NT = S // TILE              # 32 query tiles per head
NG = NT // 4                # 8 groups of 4 tiles per head
PAD = 64
SPAD = S + 2 * PAD          # 4224 padded keys
NCHUNK = SPAD // TILE       # 33 key chunks
VAW = NCHUNK * (D + 1)      # 4257 cols of rearranged augmented V
SCALE = 1.0 / math.sqrt(D)

# Packed-input chunking (consumption order). Boundaries are 512-aligned (qt),
# 128-aligned (kt) and 129-aligned (va) so every kernel slice stays inside
# one segment.
_T = [0, 4, 8, 12, 16, 20, 24, 28, 32]   # q-tile boundaries per chunk
QT_B = [128 * t for t in _T]
KT_B = [0] + [128 * t + 256 for t in _T[1:-1]] + [SPAD]
VA_B = [0] + [129 * (t + 1) for t in _T[1:-1]] + [VAW]
NCK = len(QT_B) - 1
QT_W = [QT_B[i + 1] - QT_B[i] for i in range(NCK)]
KT_W = [KT_B[i + 1] - KT_B[i] for i in range(NCK)]
VA_W = [VA_B[i + 1] - VA_B[i] for i in range(NCK)]
CHUNK_W = [QT_W[i] + KT_W[i] + VA_W[i] for i in range(NCK)]
BASE = [0]
for i in range(NCK):
    BASE.append(BASE[-1] + CHUNK_W[i])
W_PACK = BASE[-1]

_PROGRAM = None


def _qt_off(x):
    i = bisect.bisect_right(QT_B, x) - 1
    return BASE[i] + (x - QT_B[i]), i


def _kt_off(y):
    i = bisect.bisect_right(KT_B, y) - 1
    return BASE[i] + QT_W[i] + (y - KT_B[i]), i


def _va_off(z):
    i = bisect.bisect_right(VA_B, z) - 1
    return BASE[i] + QT_W[i] + KT_W[i] + (z - VA_B[i]), i


def _build_program():
    from contextlib import ExitStack

    import concourse.mybir as mybir
    import concourse.tile as tile
    from concourse import bacc

    f16 = mybir.dt.float16
    f32 = mybir.dt.float32
    Exp = mybir.ActivationFunctionType.Exp

    nc = bacc.Bacc("TRN2", target_bir_lowering=False, debug=False)
    qkv_d = nc.declare_dram_parameter("qkv", [HPC, 128, W_PACK], f16, isOutput=False)
    out_d = nc.declare_dram_parameter("out", [HPC, 128, NT // 2 * 258], f16, isOutput=True)

    def qt_sl(sb, x0, w):
        off, i = _qt_off(x0)
        assert x0 + w <= QT_B[i + 1], (x0, w)
        return sb[:, off:off + w]

    def kt_sl(sb, y0, w):
        off, i = _kt_off(y0)
        assert y0 + w <= KT_B[i + 1], (y0, w)
        return sb[:, off:off + w]

    def va_sl(sb, z0, w):
        off, i = _va_off(z0)
        assert z0 + w <= VA_B[i + 1], (z0, w)
        return sb[:, off:off + w]

    with tile.TileContext(nc) as tc, ExitStack() as ctx:
        io_pool = ctx.enter_context(tc.tile_pool(name="io", bufs=2))
        out_pool = ctx.enter_context(tc.tile_pool(name="outp", bufs=2))
        p_pool = ctx.enter_context(tc.tile_pool(name="p", bufs=4))
        ps_pool = ctx.enter_context(tc.tile_pool(name="ps", bufs=3, space="PSUM"))
        po_pool = ctx.enter_context(tc.tile_pool(name="po", bufs=2, space="PSUM"))

        # Group (0,0)'s score tile doubles as the warmup/touch scratch target:
        # its first real MM1 (start=True) clears whatever the scratch writes
        # left behind, so no dedicated PSUM bank is burned on scratch.
        ps00 = ps_pool.tile([128, 1024], f32, tag="ps")

        # PE warmup sized to fit inside the pre-data window: sustained PE
        # activity flips the HAM clock gate to 2.4 GHz early and reliably.
        warm_pool = ctx.enter_context(tc.tile_pool(name="warm", bufs=1))
        warm = warm_pool.tile([128, 512], f16, tag="warm")
        nc.gpsimd.memset(warm[:], 0.0)
        for _ in range(4):
            nc.tensor.matmul(
                ps00[0:1, 0:512], lhsT=warm[:, 0:1], rhs=warm[:],
                start=True, stop=True,
            )

        # Load phase: ALL input DMAs (both heads) are emitted first so they
        # outrank output DMAs in scheduler priority.
        io_sbs = []
        for h in range(HPC):
            io_sb = io_pool.tile([128, W_PACK], f16, tag="io")
            io_sbs.append(io_sb)
        for h in range(HPC):
            io_sb = io_sbs[h]
            for c in range(NCK):
                nc.sync.dma_start(
                    io_sb[:, BASE[c]:BASE[c + 1]], qkv_d[h, :, BASE[c]:BASE[c + 1]]
                )

        # Compute phase, software-pipelined EMISSION order by group of 4
        # tiles: group n+1's MM1 block is emitted before group n's
        # exp/memset/MM2/copy so the PE produces scores one group ahead and
        # the ACT engine never starves.
        groups = [(h, g) for h in range(HPC) for g in range(NG)]
        out_sbs = {}
        ps_tiles = {}

        def emit_mm1(h, g):
            io_sb = io_sbs[h]
            if (h, g) == (0, 0):
                ps = ps00
            else:
                ps = ps_pool.tile([128, 1024], f32, tag="ps")
            ps_tiles[(h, g)] = ps
            t0 = 4 * g
            # chunk-ordered: interior chunks do one 256-col matmul over two
            # adjacent q-tiles with the same stationary kT chunk; the chunk
            # at the PSUM bank boundary (4g+2) splits into two 128-col MMs.
            nc.tensor.matmul(
                ps[:, 0:128], lhsT=kt_sl(io_sb, 128 * t0, 128),
                rhs=qt_sl(io_sb, 128 * t0, 128), start=True, stop=True,
            )
            nc.tensor.matmul(
                ps[:, 128:384], lhsT=kt_sl(io_sb, 128 * (t0 + 1), 128),
                rhs=qt_sl(io_sb, 128 * t0, 256), start=True, stop=True,
            )
            nc.tensor.matmul(
                ps[:, 384:512], lhsT=kt_sl(io_sb, 128 * (t0 + 2), 128),
                rhs=qt_sl(io_sb, 128 * (t0 + 1), 128), start=True, stop=True,
            )
            nc.tensor.matmul(
                ps[:, 512:640], lhsT=kt_sl(io_sb, 128 * (t0 + 2), 128),
                rhs=qt_sl(io_sb, 128 * (t0 + 2), 128), start=True, stop=True,
            )
            nc.tensor.matmul(
                ps[:, 640:896], lhsT=kt_sl(io_sb, 128 * (t0 + 3), 128),
                rhs=qt_sl(io_sb, 128 * (t0 + 2), 256), start=True, stop=True,
            )
            nc.tensor.matmul(
                ps[:, 896:1024], lhsT=kt_sl(io_sb, 128 * (t0 + 4), 128),
                rhs=qt_sl(io_sb, 128 * (t0 + 3), 128), start=True, stop=True,
            )
            # PE "touch" of the NEXT packing segment, emitted AFTER this
            # group's real MM1s so they don't wait on segment g+1: a 1-col
            # dummy matmul makes PE observe that segment's DMA semaphore,
            # keeping group g+1's matmuls within the 2-sync-wait HW limit
            # without a dedicated scratch bank. It lands in ps[0, 64] --
            # inside tile t0's disallowed A-corner, which the post-exp
            # memset zeroes -- so the garbage never reaches MM2. (Its
            # start=True clears bank-0 has_written bits, which is harmless:
            # no later matmul accumulates into this ps tile.)
            if g + 1 < NCK:
                b1 = BASE[g + 1]
                nc.tensor.matmul(
                    ps[0:1, 64:65], lhsT=io_sb[:, b1:b1 + 1],
                    rhs=io_sb[:, b1:b1 + 1], start=True, stop=True,
                )

        def emit_tail(h, g):
            io_sb = io_sbs[h]
            out_sb = out_sbs[h]
            ps = ps_tiles.pop((h, g))
            p_sb = p_pool.tile([128, 1024], f16, tag="p")
            nc.scalar.activation(p_sb[:], ps[:], Exp, bias=0.0, scale=SCALE)
            # Kill disallowed 64x64 corners (cols 256*tl+0:128 = chunk A of
            # tile t, 256*tl+128:256 = chunk B); boundary tiles kill the
            # whole 64-row pad block instead. One strided memset covers all
            # four tiles' A-corners, another the four B-corners, so the
            # GpSimd stream never paces the MM2 stream.
            p3 = p_sb[:].rearrange("p (t w) -> p t w", t=4)
            # A-corners on GpSimd, B-corners on DVE: the two memsets run in
            # parallel after the exp instead of serializing on one engine,
            # halving the exp->memset->MM2 latency in each group's chain.
            nc.gpsimd.memset(p3[0:64, :, 64:128], 0.0)
            nc.vector.memset(p3[64:128, :, 128:192], 0.0)
            if g == 0:
                nc.gpsimd.memset(p_sb[0:64, 0:64], 0.0)      # tile 0 pad block
            if g == NG - 1:
                nc.vector.memset(p_sb[64:128, 960:1024], 0.0)  # last tile pad
            po_a = po_pool.tile([128, 258], f32, tag="po")
            po_b = po_pool.tile([128, 258], f32, tag="po")
            pos = [po_a, po_b]
            for tl in range(4):
                t = 4 * g + tl
                po = pos[tl // 2]
                o0 = 129 * (tl % 2)
                nc.tensor.matmul(
                    po[:, o0:o0 + 129],
                    lhsT=p_sb[:, 256 * tl:256 * tl + 128],
                    rhs=va_sl(io_sb, 129 * t, 129),
                    start=True, stop=False,
                )
                nc.tensor.matmul(
                    po[:, o0:o0 + 129],
                    lhsT=p_sb[:, 256 * tl + 128:256 * tl + 256],
                    rhs=va_sl(io_sb, 129 * (t + 1), 129),
                    start=False, stop=True,
                )
            for i in range(2):
                u = 2 * g + i
                nc.vector.tensor_copy(out_sb[:, u * 258:(u + 1) * 258], pos[i][:])
            # Stream the output back as groups complete; the final chunk is
            # small so the trailing DMA after the last pair's compute is
            # short. Activation-ring triggers: measured faster than sharing
            # the Sync ring with the input-trigger stream.
            last = h == HPC - 1
            if g == 3:
                c0, c1 = 0, 8 * 258
                nc.sync.dma_start(out_d[h, :, c0:c1], out_sb[:, c0:c1])
            elif g == 6:
                c0, c1 = 8 * 258, 14 * 258
                nc.sync.dma_start(out_d[h, :, c0:c1], out_sb[:, c0:c1])
            elif g == 7:
                if last:
                    c0, c1 = 14 * 258, 15 * 258
                    nc.sync.dma_start(out_d[h, :, c0:c1], out_sb[:, c0:c1])
                    c0, c1 = 15 * 258, 16 * 258
                    nc.sync.dma_start(out_d[h, :, c0:c1], out_sb[:, c0:c1])
                else:
                    c0, c1 = 14 * 258, 16 * 258
                    nc.sync.dma_start(out_d[h, :, c0:c1], out_sb[:, c0:c1])

        DEPTH = 2
        for n in range(len(groups) + DEPTH):
            if n < len(groups):
                h, g = groups[n]
                if g == 0:
                    out_sb = out_pool.tile([128, NT // 2 * 258], f16, tag="out")
                    out_sbs[h] = out_sb
                emit_mm1(h, g)
            if n >= DEPTH:
                emit_tail(*groups[n - DEPTH])

    nc.finalize()
    return nc


def _get_program():
    global _PROGRAM
    if _PROGRAM is None:
        _PROGRAM = _build_program()
    return _PROGRAM


def _pack_inputs(q, k, v):
    """q,k,v: [H, S, D] fp32 -> packed [H, 128, W_PACK] fp16 per head."""
    qt = np.ascontiguousarray(q.transpose(0, 2, 1)).astype(np.float16)  # [H,128,S]
    k_pad = np.zeros((H, SPAD, D), np.float32)
    k_pad[:, PAD:PAD + S] = k
    kt = np.ascontiguousarray(k_pad.transpose(0, 2, 1)).astype(np.float16)
    v_aug = np.zeros((H, SPAD, D + 1), np.float32)
    v_aug[:, PAD:PAD + S, :D] = v
    v_aug[:, :, D] = 1.0
    va = np.ascontiguousarray(
        v_aug.reshape(H, NCHUNK, 128, D + 1).transpose(0, 2, 1, 3)
    ).reshape(H, 128, VAW).astype(np.float16)
    segs = []
    for c in range(NCK):
        segs.append(qt[:, :, QT_B[c]:QT_B[c + 1]])
        segs.append(kt[:, :, KT_B[c]:KT_B[c + 1]])
        segs.append(va[:, :, VA_B[c]:VA_B[c + 1]])
    return np.ascontiguousarray(np.concatenate(segs, axis=2))


def kernel(q, k, v):
    """q, k, v: [1, 16, 4096, 128] float32 -> [1, 16, 4096, 128] float32."""
    from concourse.bass_utils import run_bass_kernel_spmd

    q = np.asarray(q, dtype=np.float32).reshape(H, S, D)
    k = np.asarray(k, dtype=np.float32).reshape(H, S, D)
    v = np.asarray(v, dtype=np.float32).reshape(H, S, D)

    qkv = _pack_inputs(q, k, v)
    in_maps = [
        {"qkv": np.ascontiguousarray(qkv[c * HPC:(c + 1) * HPC])}
        for c in range(N_CORES)
    ]

    nc = _get_program()
    results = run_bass_kernel_spmd(nc, in_maps, list(range(N_CORES))).results

    out = np.empty((H, S, D), np.float32)
    for c in range(N_CORES):
        o = results[c]["out"]  # [HPC, 128, 16*258] fp16: per pair [PV_A|den_A|PV_B|den_B]
        for j in range(HPC):
            x = o[j].astype(np.float32).reshape(128, NT, D + 1)  # [p, t, 129]
            pv = x[:, :, :D] / x[:, :, D:D + 1]     # normalize on host
            out[c * HPC + j] = pv.transpose(1, 0, 2).reshape(S, D)
    return out.reshape(B, H, S, D)
